# revision 36
# baseline (speedup 1.0000x reference)
"""Trainium2 Bass kernel: dense-CRF mean-field layer (96x96 image, 21 labels).

Strategy (8 NeuronCores, row-sharded):
  * Bilateral kernel K_bl [N,N] is built once on-device (fused feature matmul +
    exp) in bf16 and stays SBUF-resident per core as its [all j, own i] slice.
  * Spatial kernel is exactly separable (K_sp = A_y (x) A_x); it is applied as
    an x-blur (pre-all-gather, per core) + y-blur (post-gather), with the
    W_SPATIAL/norm_sp scaling folded into the host-prepared blur matrices.
  * Per iteration: P_bl[l,i] = sum_j q[j,l] K[j,i] accumulated over 72
    128-row chunks (q is the stationary operand, K streams); norm_bl comes for
    free from a 22nd "0.1" channel of q (rec = 1/(0.1 norm) = 10/norm).
  * q + x-blurred q are all-gathered between iterations (one bf16/f32 mixed
    payload via bitcast).
"""
import sys
sys.path.insert(0, "/opt/trn_rl_repo")
import os
import numpy as np
import ml_dtypes

H = W = 96
N = H * W                  # 9216
L = 21
LE = L + 1                 # 22 channels (21 labels + norm channel)
ALPHA, BETA, GAMMA = 80.0, 13.0, 3.0
W_SPATIAL, W_BILATERAL = 3.0, 10.0
NUM_ITERATIONS = 5
NCORES = 8
S = N // NCORES            # 1152 rows per core
YPC = H // NCORES          # 12 image rows per core
CH = N // 128              # 72 chunks of 128 rows (global j)
KCOLS = CH * S             # 82944 K_bl sbuf columns (bf16)
QCOLS = CH * LE            # 1584
PAYQ_F32 = 128 * 9 * LE // 2   # 12672 f32 slots holding the bf16 q-part
PAYT = S * LE // 2             # 12672 f32 slots holding the bf16 t1 part
PAY = PAYQ_F32 + PAYT          # 25344
ONESV = 0.1                # q norm-channel value => reciprocal gives 10/norm

MM_SLICES = [(0, 512), (512, 512), (1024, 128)]
T2_SLICES = [(0, 512), (512, 512), (1024, 512), (1536, 512), (2048, 64)]

LAST_EXEC_NS = None
_CACHE = {}


def _xpieces(y):
    """Split x-blur output cols [y*96, y*96+96) at PSUM 512-boundaries."""
    col = y * W
    cuts = sorted({0, W} | {b - col for b in (512, 1024) if col < b < col + W})
    return [(cuts[i], cuts[i + 1] - cuts[i]) for i in range(len(cuts) - 1)]


def _build_bass(sim1=False):
    """Build the kernel. sim1=True builds a single-core variant where the
    AllGather is replaced by 8 local DRAM copies (for TimelineSim analysis)."""
    key = "nc_sim1" if sim1 else "nc"
    if key in _CACHE:
        return _CACHE[key]
    import concourse.bass as bass  # noqa: F401
    from concourse import bacc
    import concourse.mybir as mybir
    import concourse.tile as tile

    f32 = mybir.dt.float32
    bf16 = mybir.dt.bfloat16
    AF = mybir.ActivationFunctionType
    OP = mybir.AluOpType
    AX = mybir.AxisListType

    dbg = bool(int(os.environ.get("CRF_DEBUG", "0"))) and not sim1
    nc = bacc.Bacc("TRN2", target_bir_lowering=False, debug=False,
                   num_devices=1 if sim1 else NCORES)

    featL_d = nc.dram_tensor("featL", [21, N], bf16, kind="ExternalInput")
    featR_d = nc.dram_tensor("featR", [21, S], bf16, kind="ExternalInput")
    uT_d = nc.dram_tensor("uT", [LE, S], f32, kind="ExternalInput")
    Ax_d = nc.dram_tensor("Ax", [W, W], bf16, kind="ExternalInput")
    Ay_d = nc.dram_tensor("Ay", [H, YPC], bf16, kind="ExternalInput")
    qsb0_d = nc.dram_tensor("qsb0", [128, QCOLS], bf16, kind="ExternalInput")
    t1f0_d = nc.dram_tensor("t1f0", [H, LE * W], bf16, kind="ExternalInput")
    id_d = nc.dram_tensor("ident", [LE, LE], f32, kind="ExternalInput")
    qout_d = nc.dram_tensor("qout", [S, L], f32, kind="ExternalOutput")
    if dbg:
        dbg_kbl = nc.dram_tensor("dbg_kbl", [128, S], bf16, kind="ExternalOutput")
        dbg_pbl = nc.dram_tensor("dbg_pbl", [LE, S], f32, kind="ExternalOutput")
        dbg_usp = nc.dram_tensor("dbg_usp", [LE, S], f32, kind="ExternalOutput")
        dbg_qy = nc.dram_tensor("dbg_qy", [W, YPC * LE], f32, kind="ExternalOutput")
        dbg_t1t = nc.dram_tensor("dbg_t1t", [LE, S], bf16, kind="ExternalOutput")
        dbg_q128 = nc.dram_tensor("dbg_q128", [128, 9 * LE], bf16, kind="ExternalOutput")
        dbg_t1full = nc.dram_tensor("dbg_t1full", [H, LE * W], bf16, kind="ExternalOutput")

    with tile.TileContext(nc) as tc:
        with (
            tc.tile_pool(name="const", bufs=1) as constp,
            tc.tile_pool(name="kbl", bufs=1) as kblp,
            tc.tile_pool(name="work", bufs=1) as work,
            tc.tile_pool(name="dram", bufs=2, space="DRAM") as dram,
        ):
            Ax = constp.tile([W, W], bf16)
            nc.sync.dma_start(Ax[:], Ax_d[:])
            Ay = constp.tile([H, YPC], bf16)
            nc.sync.dma_start(Ay[:], Ay_d[:])
            uT = constp.tile([LE, S], f32)
            nc.sync.dma_start(uT[:], uT_d[:])
            idn = constp.tile([LE, LE], f32)
            nc.sync.dma_start(idn[:], id_d[:])
            Kbl = kblp.tile([128, KCOLS], bf16)

            # psA holds P_bl + the y-blur slot; opened before the precompute
            # psum pool so iteration-1's bilateral burst can run inside the
            # (ACT-bound) precompute window without bank WAR serialization.
            psA_ctx = tc.tile_pool(name="psA", bufs=1, space="PSUM")
            psA = psA_ctx.__enter__()

            # ---------- precompute K_bl = exp(-||g_i - g_j||^2 / 2) ----------
            with (
                tc.tile_pool(name="pre_sb", bufs=2) as pre_sb,
                tc.tile_pool(name="featRp", bufs=1) as featRp,
                tc.tile_pool(name="pre_ps", bufs=2, space="PSUM") as pre_ps,
            ):
                featR = featRp.tile([21, S], bf16)
                nc.sync.dma_start(featR[:], featR_d[:])
                WIN = 1024
                flb, flb_idx = None, -1
                X = 0
                while X < KCOLS:
                    wlen = min(WIN, KCOLS - X)
                    d2 = pre_ps.tile([128, WIN], f32, tag="d2")
                    cuts = sorted({X, X + wlen}
                                  | set(range((X // 512 + 1) * 512,
                                              X + wlen, 512))
                                  | set(range((X // S + 1) * S, X + wlen, S)))
                    for a, b in zip(cuts[:-1], cuts[1:]):
                        ch = a // S
                        if ch // 8 != flb_idx:
                            flb_idx = ch // 8
                            flb = pre_sb.tile([21, 1024], bf16, tag="fl")
                            nc.sync.dma_start(
                                flb[:],
                                featL_d[:, flb_idx * 1024:(flb_idx + 1) * 1024])
                        ci = ch - flb_idx * 8
                        nc.tensor.matmul(d2[:, a - X: b - X],
                                         flb[:, ci * 128:(ci + 1) * 128],
                                         featR[:, a - ch * S: b - ch * S],
                                         start=True, stop=True)
                    nc.scalar.activation(Kbl[:, X:X + wlen], d2[:, 0:wlen],
                                         AF.Exp)
                    X += wlen

            if dbg:
                nc.sync.dma_start(dbg_kbl.ap(), Kbl[:, 0:S])

            # ---------- mean-field iterations ----------
            psum_ctx = tc.tile_pool(name="psum", bufs=1, space="PSUM")
            psum = psum_ctx.__enter__()
            qag_prev = None
            for it in range(NUM_ITERATIONS):
                last = it == NUM_ITERATIONS - 1
                qsb = work.tile([128, QCOLS], bf16, tag="qsb")
                t1full = work.tile([H, LE * W], bf16, tag="t1full")
                if it == 0:
                    nc.sync.dma_start(qsb[:], qsb0_d[:])
                    nc.sync.dma_start(t1full[:], t1f0_d[:])

                else:
                    for r in range(NCORES):
                        qsrc = (qag_prev[r:r + 1, 0:PAYQ_F32].bitcast(bf16)
                                .rearrange("a (p c) -> (a p) c", p=128))
                        eng = nc.sync if r % 2 else nc.scalar
                        eng.dma_start(qsb[:, r * 9 * LE:(r + 1) * 9 * LE],
                                      qsrc)
                        tsrc = (qag_prev[r:r + 1, PAYQ_F32:PAY].bitcast(bf16)
                                .rearrange("a b -> (a b)")
                                .rearrange("(l y x) -> y l x",
                                           l=LE, y=YPC, x=W))
                        tdst = (t1full[r * YPC:(r + 1) * YPC, :]
                                .rearrange("y (l x) -> y l x", l=LE, x=W))
                        eng = nc.scalar if r % 2 else nc.sync
                        eng.dma_start(tdst, tsrc)

                if dbg and it == 1:
                    nc.sync.dma_start(dbg_t1full.ap(), t1full[:])

                # spatial y-blur first (small PE work; its DRAM round-trip
                # and the u+sp add then overlap with the bilateral burst)
                tb = work.tile([YPC, LE * W], f32, tag="t2b")
                t2scr = dram.tile([YPC, LE * W], f32, tag="t2scr")
                for (o, w) in T2_SLICES:
                    t2p = psA.tile([YPC, 512], f32, tag="yb", bufs=1)
                    nc.tensor.matmul(t2p[:, 0:w], Ay[:], t1full[:, o:o + w],
                                     start=True, stop=True)
                    nc.scalar.copy(tb[:, o:o + w], t2p[:, 0:w])
                nc.scalar.dma_start(t2scr[:], tb[:])
                t2T = work.tile([LE, YPC * W], f32, tag="t2T")
                nc.sync.dma_start(
                    t2T[:].rearrange("l (y x) -> l y x", y=YPC, x=W),
                    t2scr[:].rearrange("y (l x) -> l y x", l=LE, x=W),
                )
                nc.vector.tensor_add(t2T[:], t2T[:], uT[:])

                # bilateral message, grouped by output y-range so the
                # combine/softmax tail pipelines into the burst:
                #   g0: y 0-4  (cols    0:480), g1: y 5-9 (480:960),
                #   g2: y 10-11 (960:1152)
                P_bl = psA.tile([LE, S], f32, tag="A")
                pbs = work.tile([LE, S], f32, tag="t1Ts")
                qy = work.tile([W, YPC * LE], f32, tag="qy")
                qy3 = qy[:].rearrange("x (y l) -> x y l", y=YPC, l=LE)
                ssum = work.tile([W, YPC], f32, tag="ssum")
                rec12 = work.tile([W, YPC], f32, tag="rec12")
                blsc = work.tile([W, YPC * LE], f32, tag="blsc")
                blsc3 = blsc[:].rearrange("x (y l) -> x y l", y=YPC, l=LE)
                if not last:
                    qyb = work.tile([W, YPC * LE], bf16, tag="qyb")
                    t1Ts = work.tile([LE, S], bf16, tag="t1Tsb")
                GROUPS = (
                    (0, 5, ((0, 480),)),
                    (5, 10, ((480, 32), (512, 448))),
                    (10, 12, ((960, 64), (1024, 128))),
                )
                for gi, (y0, y1, pieces) in enumerate(GROUPS):
                    c0, c1 = y0 * W, y1 * W
                    for ch in range(CH):
                        lhs = qsb[:, ch * LE:(ch + 1) * LE]
                        for (o, w) in pieces:
                            nc.tensor.matmul(P_bl[:, o:o + w], lhs,
                                             Kbl[:, ch * S + o: ch * S + o + w],
                                             start=(ch == 0), stop=(ch == CH - 1))
                    nc.scalar.copy(pbs[:, c0:c1], P_bl[:, c0:c1])
                    tp = psum.tile([W, 2 * 5 * LE], f32, tag="tp", bufs=2)
                    ng = y1 - y0
                    for k, y in enumerate(range(y0, y1)):
                        nc.tensor.transpose(tp[:, k * LE:(k + 1) * LE],
                                            t2T[:, y * W:(y + 1) * W], idn[:])
                        nc.tensor.transpose(
                            tp[:, (5 + k) * LE:(5 + k + 1) * LE],
                            pbs[:, y * W:(y + 1) * W], idn[:])
                    tp0 = tp[:, 0:ng * LE].rearrange("x (y l) -> x y l", l=LE)
                    tpB = tp[:, 5 * LE:(5 + ng) * LE].rearrange(
                        "x (y l) -> x y l", l=LE)
                    nc.vector.reciprocal(rec12[:, y0:y1][:, :, None],
                                         tpB[:, :, L:LE])
                    nc.vector.tensor_tensor(
                        blsc3[:, y0:y1], tpB,
                        rec12[:, y0:y1][:, :, None].to_broadcast([W, ng, LE]),
                        OP.mult)
                    nc.vector.tensor_tensor(tp0, tp0, blsc3[:, y0:y1], OP.add)
                    nc.scalar.activation(qy[:, y0 * LE:y1 * LE],
                                         tp[:, 0:ng * LE], AF.Exp)
                    nc.vector.reduce_sum(ssum[:, y0:y1],
                                         qy3[:, y0:y1, 0:L], axis=AX.X,)
                    nc.vector.reciprocal(ssum[:, y0:y1], ssum[:, y0:y1])
                    nc.vector.tensor_tensor(
                        qy3[:, y0:y1, 0:L], qy3[:, y0:y1, 0:L],
                        ssum[:, y0:y1][:, :, None].to_broadcast([W, ng, L]),
                        OP.mult)
                    if last:
                        nc.sync.dma_start(
                            qout_d.ap()[y0 * W:y1 * W, :]
                                 .rearrange("(y x) l -> x y l", x=W),
                            qy3[:, y0:y1, 0:L])
                    if not last:
                        nc.vector.memset(qy3[:, y0:y1, L:LE], ONESV)
                        nc.vector.tensor_copy(qyb[:, y0 * LE:y1 * LE],
                                              qy[:, y0 * LE:y1 * LE])
                        # x-blur this group's rows; evacuate to bf16 per piece
                        for y in range(y0, y1):
                            for (xo, xw) in _xpieces(y):
                                xb = psum.tile([LE, 512], f32, tag="xb", bufs=2)
                                nc.tensor.matmul(
                                    xb[:, 0:xw],
                                    qyb[:, y * LE:(y + 1) * LE],
                                    Ax[:, xo:xo + xw],
                                    start=True, stop=True)
                                nc.scalar.copy(
                                    t1Ts[:, y * W + xo: y * W + xo + xw],
                                    xb[:, 0:xw])

                if last:
                    continue

                if dbg and it == 0:
                    nc.sync.dma_start(dbg_pbl.ap(), pbs[:])
                    nc.sync.dma_start(dbg_usp.ap(), t2T[:])
                    nc.sync.dma_start(dbg_qy.ap(), qy[:])
                    nc.sync.dma_start(dbg_t1t.ap(), t1Ts[:])

                # payload: [bf16 q chunk-major | bf16 t1T] then AllGather
                q128 = work.tile([128, 9 * LE], bf16, tag="q128")
                qyb4 = qyb[:].rearrange("x (yi yo l) -> x yo yi l",
                                        yi=3, yo=4, l=LE)
                q128r = q128[:].rearrange("p (ai ao l) -> p ao ai l",
                                          ai=3, ao=3, l=LE)
                # (x0, n, p0, yo, ao): partition-shift piece groups
                for (x0, n, p0, yo, ao) in ((0, 96, 0, 0, 0),
                                            (0, 96, 32, 3, 2),
                                            (0, 32, 96, 1, 0),
                                            (32, 64, 0, 1, 1),
                                            (0, 64, 64, 2, 1),
                                            (64, 32, 0, 2, 2)):
                    nc.scalar.dma_start(q128r[p0:p0 + n, ao],
                                        qyb4[x0:x0 + n, yo])

                pl = dram.tile([1, PAY], f32, tag="pl")
                nc.sync.dma_start(
                    pl[0:1, 0:PAYQ_F32].bitcast(bf16)
                      .rearrange("a (p c) -> (a p) c", p=128),
                    q128[:])
                nc.scalar.dma_start(
                    pl[0:1, PAYQ_F32:PAY].bitcast(bf16)
                      .rearrange("a (l c) -> (a l) c", l=LE),
                    t1Ts[:])
                if dbg and it == 0:
                    nc.sync.dma_start(dbg_q128.ap(), q128[:])
                qag = dram.tile([NCORES, PAY], f32, tag="qag")
                if sim1:
                    for r in range(NCORES):
                        nc.sync.dma_start(qag[r:r + 1, :], pl[:])
                else:
                    nc.gpsimd.collective_compute(
                        "AllGather", OP.bypass,
                        replica_groups=[list(range(NCORES))],
                        ins=[pl.opt()], outs=[qag.opt()])
                qag_prev = qag
            psum_ctx.__exit__(None, None, None)
            psA_ctx.__exit__(None, None, None)

    nc.compile()
    _CACHE[key] = nc
    return nc


def _host_prepare(unaries, rgb):
    u = np.asarray(unaries, np.float32).reshape(N, L)
    c = np.asarray(rgb, np.float32).reshape(N, 3)

    ys, xs = np.meshgrid(np.arange(H, dtype=np.float64),
                         np.arange(W, dtype=np.float64), indexing="ij")
    pos = np.stack([ys.ravel(), xs.ravel()], -1)            # [N, 2]
    g = np.concatenate([c.astype(np.float64) / BETA, pos / ALPHA], 1)
    g = g - g.mean(0, keepdims=True)
    sq = (g * g).sum(1)
    ones = np.ones(N, np.float64)
    L7 = np.concatenate([g.T, ones[None], (-0.5 * sq)[None]], 0)  # [7, N] j
    R7 = np.concatenate([g.T, (-0.5 * sq)[None], ones[None]], 0)  # [7, N] i
    bfd = ml_dtypes.bfloat16
    Lhi = L7.astype(bfd)
    Llo = (L7 - Lhi.astype(np.float64)).astype(bfd)
    Rhi = R7.astype(bfd)
    Rlo = (R7 - Rhi.astype(np.float64)).astype(bfd)
    # dot = Lhi.Rhi + Lhi.Rlo + Llo.Rhi  (Llo.Rlo dropped, ~1e-3)
    featL = np.ascontiguousarray(np.concatenate([Lhi, Lhi, Llo], 0))  # [21,N]
    featR = np.ascontiguousarray(np.concatenate([Rhi, Rlo, Rhi], 0))  # [21,N]

    d = np.arange(W, dtype=np.float64)
    A = np.exp(-(d[:, None] - d[None, :]) ** 2 / (2.0 * GAMMA * GAMMA))
    nvec = A.sum(0)
    Ax = np.ascontiguousarray((A / nvec[None, :]).astype(ml_dtypes.bfloat16))

    um = u.max(1, keepdims=True)
    e = np.exp(u - um)
    q0 = e / e.sum(1, keepdims=True)
    q0e = np.concatenate([q0, np.full((N, 1), ONESV, np.float32)], 1)  # [N,22]
    qsb0 = np.ascontiguousarray(
        q0e.reshape(CH, 128, LE).transpose(1, 0, 2).reshape(128, QCOLS)
    ).astype(ml_dtypes.bfloat16)

    q3 = q0e.reshape(H, W, LE).astype(np.float64)
    t1 = np.einsum("Xx,yXl->ylx", A / nvec[None, :], q3)      # [96, 22, 96]
    t1f0 = np.ascontiguousarray(t1.reshape(H, LE * W).astype(ml_dtypes.bfloat16))

    ident = np.eye(LE, dtype=np.float32)

    in_maps = []
    for core in range(NCORES):
        rows = slice(core * S, (core + 1) * S)
        uT_c = np.full((LE, S), -50.0, np.float32)
        uT_c[0:L] = u[rows].T
        yc = slice(core * YPC, (core + 1) * YPC)
        Ay_c = np.ascontiguousarray(
            (A[:, yc] * (W_SPATIAL / nvec[yc])[None, :]).astype(ml_dtypes.bfloat16))
        in_maps.append({
            "featL": featL,
            "featR": np.ascontiguousarray(featR[:, rows]),
            "uT": uT_c,
            "Ax": Ax,
            "Ay": Ay_c,
            "qsb0": qsb0,
            "t1f0": t1f0,
            "ident": ident,
        })
    return in_maps


def _get_runner():
    """Compile once; return (fn, in_names, out_names) where fn maps
    concatenated global numpy inputs -> list of per-core output dicts."""
    if "runner" in _CACHE:
        return _CACHE["runner"]
    import jax
    from jax.sharding import Mesh, PartitionSpec
    from jax.experimental.shard_map import shard_map
    import concourse.mybir as mybir
    from concourse import bass2jax

    nc = _build_bass()
    bass2jax.install_neuronx_cc_hook()

    partition_name = (nc.partition_id_tensor.name
                      if nc.partition_id_tensor else None)
    in_names, out_names, out_avals, zero_outs = [], [], [], []
    for alloc in nc.m.functions[0].allocations:
        if not isinstance(alloc, mybir.MemoryLocationSet):
            continue
        name = alloc.memorylocations[0].name
        if alloc.kind == "ExternalInput":
            if name != partition_name:
                in_names.append(name)
        elif alloc.kind == "ExternalOutput":
            shape = tuple(alloc.tensor_shape)
            dtype = mybir.dt.np(alloc.dtype)
            out_names.append(name)
            out_avals.append(jax.core.ShapedArray(shape, dtype))
            zero_outs.append(np.zeros(shape, dtype))
    n_params = len(in_names)
    all_in_names = list(in_names) + list(out_names)
    if partition_name is not None:
        all_in_names.append(partition_name)

    def _body(*args):
        operands = list(args)
        if partition_name is not None:
            operands.append(bass2jax.partition_id_tensor())
        outs = bass2jax._bass_exec_p.bind(
            *operands,
            out_avals=tuple(out_avals),
            in_names=tuple(all_in_names),
            out_names=tuple(out_names),
            lowering_input_output_aliases=(),
            sim_require_finite=False,
            sim_require_nnan=False,
            nc=nc,
        )
        return tuple(outs)

    devices = jax.devices()[:NCORES]
    mesh = Mesh(np.asarray(devices), ("core",))
    n_outs = len(out_names)
    in_specs = (PartitionSpec("core"),) * (n_params + n_outs)
    out_specs = (PartitionSpec("core"),) * n_outs
    donate = tuple(range(n_params, n_params + n_outs))
    fn = jax.jit(
        shard_map(_body, mesh=mesh, in_specs=in_specs, out_specs=out_specs,
                  check_rep=False),
        donate_argnums=donate, keep_unused=True)
    _CACHE["runner"] = (fn, in_names, out_names, out_avals, zero_outs)
    return _CACHE["runner"]


def _concat_inputs(in_maps, in_names):
    return [np.concatenate([np.asarray(in_maps[c][nm]) for c in range(NCORES)],
                           axis=0) for nm in in_names]


def _run(in_maps):
    fn, in_names, out_names, out_avals, zero_outs = _get_runner()
    concat_in = _concat_inputs(in_maps, in_names)
    concat_zeros = [np.zeros((NCORES * z.shape[0], *z.shape[1:]), z.dtype)
                    for z in zero_outs]
    out_arrs = fn(*concat_in, *concat_zeros)
    return out_arrs, out_names, out_avals


def kernel(unaries, rgb):
    in_maps = _host_prepare(unaries, rgb)
    out_arrs, out_names, out_avals = _run(in_maps)
    qi = out_names.index("qout")
    q = np.asarray(out_arrs[qi]).reshape(NCORES, S, L).reshape(N, L)
    return np.ascontiguousarray(q[None].astype(np.float32))


def time_kernel(unaries, rgb, iters=20):
    """Steady-state per-call wall time of the compiled 8-core executable,
    with inputs pre-staged on device."""
    import time as _time
    import jax
    in_maps = _host_prepare(unaries, rgb)
    fn, in_names, out_names, out_avals, zero_outs = _get_runner()
    concat_in = _concat_inputs(in_maps, in_names)

    def once():
        concat_zeros = [np.zeros((NCORES * z.shape[0], *z.shape[1:]), z.dtype)
                        for z in zero_outs]
        outs = fn(*concat_in, *concat_zeros)
        jax.block_until_ready(outs)
        return outs

    once()  # warm
    times = []
    for _ in range(iters):
        t0 = _time.perf_counter()
        once()
        times.append(_time.perf_counter() - t0)
    return min(times), sorted(times)[len(times) // 2]
